# revision 20
# baseline (speedup 1.0000x reference)
"""Trainium2 Bass kernel for the contrastive loss problem.

Math reformulation of the reference (no [N, 2N-1] scatter needed):
  lse_i = log( exp(pos_val_i) + sum_{j in neg} exp(S_ij) + (2N-2-num_neg_i) )
  loss  = mean_i (lse_i - pos_val_i)
with S = (cos + 1) * 0.25, cos from row-normalized embeddings.

Sharding uses the Gram matrix's symmetry: core c computes only the
[512, 512*5] strip of exp(S) pairing its rows with block-columns
{c, c+1, .., c+4} (mod 8). Columns are pre-rotated on the host so the
program is identical on every core (SPMD). Row sums cover the strip;
ones-vector matmuls produce per-column sums for the foreign blocks
(distance 1..3), which the host adds to those rows' totals. Distance-4
blocks are computed by both endpoint cores (row sums only), so no
column contribution is needed for them. The matmul runs in fp8 e4m3
(DoubleRow, K=256 per op) on x16-prescaled unit rows; exp/masking on
ScalarE/VectorE per 512-wide chunk.

Host: norms, fp8/bf16 casts, rotation, first-positive gather (label
metadata), final assembly of ~4096 scalars.
"""

import sys

sys.path.insert(0, "/opt/trn_rl_repo")

from contextlib import ExitStack

import ml_dtypes
import numpy as np

import concourse.bacc as bacc
import concourse.tile as tile
from concourse import mybir
from concourse.bass_utils import run_bass_kernel_spmd

N, D = 4096, 1024
NCORES = 8
R = N // NCORES            # 512 rows per core
P = 128                    # partitions
MI = R // P                # 4 row chunks per core
KC = D // P                # 8 contraction chunks
JW = 512                   # j tile width (one PSUM bank)
NB = 5                     # block-columns per core (self + 4 right neighbors)
JCOLS = NB * JW            # 2560
EPS = 1e-8
BF16 = ml_dtypes.bfloat16
FP8 = ml_dtypes.float8_e4m3
SCALE = 16.0

_CACHE = {}


def _build_program():
    nc = bacc.Bacc("TRN2", target_bir_lowering=False, debug=False)
    f32, bf16, fp8 = mybir.dt.float32, mybir.dt.bfloat16, mybir.dt.float8e4
    AF = mybir.ActivationFunctionType
    OP = mybir.AluOpType

    et_d = nc.dram_tensor("et", [KC, P, JCOLS], fp8, kind="ExternalInput")
    yt_d = nc.dram_tensor("yt", [P, JCOLS], bf16, kind="ExternalInput")
    yb_d = nc.dram_tensor("yb", [P, MI], f32, kind="ExternalInput")
    en_d = nc.dram_tensor("en", [MI, P, D], bf16, kind="ExternalInput")
    ef_d = nc.dram_tensor("ef", [MI, P, D], bf16, kind="ExternalInput")
    ro_d = nc.dram_tensor("rowout", [P, 2 * MI], f32, kind="ExternalOutput")
    cs_d = nc.dram_tensor("csout", [96, 2 * JW], f32, kind="ExternalOutput")

    with tile.TileContext(nc) as tc, ExitStack() as ctx:
        const = ctx.enter_context(tc.tile_pool(name="const", bufs=1))
        psum = ctx.enter_context(tc.tile_pool(name="psum", bufs=6, space="PSUM"))
        cspsum = ctx.enter_context(tc.tile_pool(name="cspsum", bufs=1,
                                                space="PSUM"))
        work = ctx.enter_context(tc.tile_pool(name="work", bufs=8))
        acc = ctx.enter_context(tc.tile_pool(name="acc", bufs=2))

        et = const.tile([P, KC, JCOLS], fp8, tag="et")
        yt = const.tile([P, JCOLS], bf16, tag="yt")
        yb = const.tile([P, MI], f32, tag="yb")
        en = const.tile([P, MI, D], bf16, tag="en")
        ef = const.tile([P, MI, D], bf16, tag="ef")
        b025 = const.tile([P, 1], f32, tag="b025")
        nc.vector.memset(b025, 0.25)
        ones = const.tile([P, 1], bf16, tag="ones")
        nc.vector.memset(ones, 1.0)
        rowout = const.tile([P, 2 * MI], f32, tag="rowout")
        nsout = rowout[:, 0:MI]
        pdout = rowout[:, MI:2 * MI]
        cs1 = cspsum.tile([P, JW], f32, tag="cs1")
        cs2 = cspsum.tile([P, JW], f32, tag="cs2")
        nc.vector.memset(cs1, 0.0)
        nc.vector.memset(cs2, 0.0)
        warm = const.tile([P, 1], f32, tag="warm")
        nc.scalar.activation(warm, b025, AF.Exp, bias=b025, scale=1.0)
        # warm the PE clock gate during the initial DMA wait: ~4us of tiny
        # matmuls into a partition strip the column sums never touch
        wsrc = const.tile([P, P], bf16, tag="wsrc")
        nc.vector.memset(wsrc, 1.0)
        for _ in range(40):
            nc.tensor.matmul(
                cs1[96:97, 0:P], ones, wsrc, start=True, stop=True,
                tile_position=(0, 96), skip_group_check=True,
            )

        # All input DMAs on the sync ring, ordered for earliest PE start:
        # first two k-chunks, then labels (needed by the first mask op),
        # then the rest.
        for k in range(4):
            nc.sync.dma_start(out=et[:, k, :], in_=et_d[k])
        nc.sync.dma_start(out=yt, in_=yt_d[:])
        for k in range(4, KC):
            nc.sync.dma_start(out=et[:, k, :], in_=et_d[k])
        nc.sync.dma_start(out=yb, in_=yb_d[:])
        for m in range(MI):
            nc.sync.dma_start(out=en[:, m, :], in_=en_d[m])
            nc.sync.dma_start(out=ef[:, m, :], in_=ef_d[m])

        for m in range(MI):
            t1 = acc.tile([P, NB], f32, tag="t1")
            t2 = acc.tile([P, NB], f32, tag="t2")
            es_keep = [None] * 3
            mm_keep = [None] * 3
            for j in range(NB):
                pt = psum.tile([P, JW], f32, tag="pt")
                for k2 in range(KC // 2):
                    nc.tensor.matmul(
                        pt,
                        et[:, 2 * k2:2 * k2 + 2, m * P:(m + 1) * P],
                        et[:, 2 * k2:2 * k2 + 2, j * JW:(j + 1) * JW],
                        start=(k2 == 0),
                        stop=(k2 == KC // 2 - 1),
                        perf_mode=mybir.MatmulPerfMode.DoubleRow,
                    )
                # expS = exp(cos*0.25 + 0.25); t1[:, j] = row-sum
                es = work.tile([P, JW], bf16, tag="es")
                nc.scalar.activation(
                    es, pt, AF.Exp, bias=b025, scale=0.25 / (SCALE * SCALE),
                    accum_out=t1[:, j:j + 1],
                )
                # t2[:, j] = row-sum((y == y_row) * expS), fused mask+mult
                mm = work.tile([P, JW], bf16, tag="mm")
                nc.vector.scalar_tensor_tensor(
                    mm, yt[:, j * JW:(j + 1) * JW], yb[:, m:m + 1], es,
                    op0=OP.is_equal, op1=OP.mult,
                    accum_out=t2[:, j:j + 1],
                )
                if 1 <= j <= 3:
                    es_keep[j - 1] = es
                    mm_keep[j - 1] = mm
            # column sums for the foreign blocks, batched to limit
            # LDWEIGHTS mode switching; accumulated over the 4 row chunks
            # at partition 32*(j-1) of a shared PSUM bank.
            for jj in range(3):
                pb = 32 * jj
                nc.tensor.matmul(
                    cs1[pb:pb + 1, :], ones, es_keep[jj],
                    start=(m == 0), stop=(m == MI - 1),
                    tile_position=(0, pb), skip_group_check=True,
                )
            for jj in range(3):
                pb = 32 * jj
                nc.tensor.matmul(
                    cs2[pb:pb + 1, :], ones, mm_keep[jj],
                    start=(m == 0), stop=(m == MI - 1),
                    tile_position=(0, pb), skip_group_check=True,
                )
            # nsout[:, m] = sum_j(t1 - t2) = rows' (T1 - T2) strip partial
            d5 = acc.tile([P, NB], f32, tag="d5")
            nc.vector.scalar_tensor_tensor(
                d5, t1, 1.0, t2, op0=OP.mult, op1=OP.subtract,
                accum_out=nsout[:, m:m + 1],
            )
            # pdout[:, m] = row-wise <e_i, e_firstpos(i)>
            pdo = work.tile([P, D], bf16, tag="pdo")
            nc.vector.scalar_tensor_tensor(
                pdo, en[:, m, :], 1.0, ef[:, m, :],
                op0=OP.mult, op1=OP.mult, accum_out=pdout[:, m:m + 1],
            )
        # evict column sums (DMA cannot read PSUM); use both ACT and DVE
        csev = const.tile([P, 2 * JW], f32, tag="csev")
        nc.scalar.copy(csev[0:96, 0:JW], cs1[0:96, :])
        nc.vector.tensor_copy(csev[0:96, JW:2 * JW], cs2[0:96, :])
        nc.sync.dma_start(out=ro_d[:, :], in_=rowout)
        nc.sync.dma_start(out=cs_d[:, :], in_=csev[0:96, :])

    nc.compile()
    return nc


def _get_program():
    if "nc" not in _CACHE:
        _CACHE["nc"] = _build_program()
    return _CACHE["nc"]


def _host_prep(layer_embeds, y_true):
    E = np.asarray(layer_embeds, dtype=np.float32)
    y = np.asarray(y_true).astype(np.int32)

    norms = np.maximum(np.linalg.norm(E, axis=1), EPS).astype(np.float32)
    Ehf = E / norms[:, None]
    Eh = Ehf.astype(BF16)
    Eh8T = np.ascontiguousarray((Ehf * SCALE).astype(FP8).T)  # [D, N]

    same = y[:, None] == y[None, :]
    nsame = same.sum(1)
    haspos = nsame > 1
    np.fill_diagonal(same, False)
    fp = np.argmax(same, axis=1)                      # first positive (j order)
    yb16 = y.astype(BF16)

    in_maps = []
    for c in range(NCORES):
        r0, r1 = c * R, (c + 1) * R
        cols = np.concatenate(
            [np.arange(((c + b) % NCORES) * R, ((c + b) % NCORES) * R + R)
             for b in range(NB)])
        etc = np.ascontiguousarray(Eh8T[:, cols]).reshape(KC, P, JCOLS)
        ytc = np.ascontiguousarray(
            np.broadcast_to(yb16[cols][None, :], (P, JCOLS)))
        in_maps.append({
            "et": etc,
            "yt": ytc,
            "yb": np.ascontiguousarray(y[r0:r1].astype(np.float32)
                                       .reshape(MI, P).T),
            "en": np.ascontiguousarray(Eh[r0:r1].reshape(MI, P, D)),
            "ef": np.ascontiguousarray(Eh[fp[r0:r1]].reshape(MI, P, D)),
        })
    meta = {"haspos": haspos, "nsame": nsame, "fp": fp}
    return in_maps, meta


def _assemble(results, meta):
    """Combine per-core partials into the scalar loss (O(N) host math)."""
    haspos = meta["haspos"]
    nsame = meta["nsame"]

    neg = np.zeros(N, dtype=np.float64)   # (T1 - T2) per row
    posd = np.zeros(N, dtype=np.float64)  # <e_i, e_fp(i)>
    for c in range(NCORES):
        r = results[c]
        rows = np.arange(c * R, (c + 1) * R)
        ro = np.asarray(r["rowout"], np.float64)
        neg[rows] += ro[:, 0:MI].T.reshape(-1)
        posd[rows] += ro[:, MI:2 * MI].T.reshape(-1)
        csb = np.asarray(r["csout"], np.float64)      # [96, 2*JW]
        cs = np.stack([csb[:, 0:JW], csb[:, JW:2 * JW]])
        for d in range(1, 4):
            b = (c + d) % NCORES
            rows_b = np.arange(b * R, b * R + R)
            # partition 32*(d-1) holds the [1, 512] column sums of the
            # distance-d block; JW == R so they map 1:1 onto b's rows
            neg[rows_b] += cs[0, 32 * (d - 1), :] - cs[1, 32 * (d - 1), :]

    posS = (posd + 1.0) * 0.25
    nneg = N - nsame
    total = neg + np.where(haspos, np.exp(posS), 1.0) + (2 * N - 2 - nneg)
    posval = np.where(haspos, posS, 0.0)
    loss = float(np.mean(np.log(total) - posval))
    return np.float32(loss)


def _install_ntff_shim():
    """Provide antenv.axon_hooks (absent in this image) so trace=True works."""
    import importlib
    import types
    try:
        importlib.import_module("antenv.axon_hooks")
        return
    except ImportError:
        pass
    try:
        import antenv
        from trn_agent_boot.trn_boot import _ntff_profile_via_ctypes

        hook = _ntff_profile_via_ctypes("/opt/axon/libaxon_pjrt.so")
        mod = types.ModuleType("antenv.axon_hooks")
        mod._hook = hook
        mod.get_axon_ntff_profile_hook = lambda: mod._hook
        mod.set_axon_ntff_profile_hook = lambda h: setattr(mod, "_hook", h)
        sys.modules["antenv.axon_hooks"] = mod
        antenv.axon_hooks = mod
    except Exception as e:  # profiling is best-effort
        print(f"ntff shim failed: {e}")


def kernel(layer_embeds, y_true, _trace=False):
    import time

    if _trace:
        _install_ntff_shim()
    nc = _get_program()
    in_maps, meta = _host_prep(layer_embeds, y_true)
    last_err = None
    for attempt in range(4):
        try:
            res = run_bass_kernel_spmd(
                nc, in_maps, core_ids=list(range(NCORES)), trace=_trace,
            )
            loss = _assemble(res.results, meta)
            # lse is bounded by log(2N-2) .. log(2N + N*e^0.5) for this
            # problem shape; anything outside is transient corruption.
            if not (np.isfinite(loss) and 5.0 < float(loss) < 20.0):
                raise RuntimeError(f"implausible loss {loss}, retrying")
            if _trace:
                return loss, res
            return loss
        except Exception as e:  # transient device faults: retry
            last_err = e
            time.sleep(5 * (attempt + 1))
    raise last_err
